# revision 14
# baseline (speedup 1.0000x reference)
"""Causal self-attention (B=2, T=4096, C=768, H=12, D=64) on 8 TRN2 NeuronCores.

Sharding: tensor-parallel over heads x data-parallel over batch.
  core i (i in 0..7): batch b = i // 4, heads hs..hs+2 where hs = 3 * (i % 4).

Per-core kernel:
  1. x[b] -> bf16 -> on-chip DMA-transpose -> x^T [768, 4096]
  2. QKV^T projection for its 3 heads (bf16 matmuls, f32 PSUM accumulation)
  3. causal attention per head: S^T = K^T.T @ Q^T blocks, exp (no max-sub:
     logits are O(1) so exp is safe), diagonal-block masking, P@V' with an
     appended ones-column producing row sums, then normalization
  4. partial output projection (only its heads' rows of w_proj)
  5. ReduceScatter(add) over each 4-core group scattering T -> each core owns
     a 1024-row chunk of the final output.

Host side only shards/concatenates and pre-slices weight columns.
"""

import numpy as np

B, T, C, H, D = 2, 4096, 768, 12, 64
N_CORES = 8
HPC = 3            # heads per core
QCH = 512          # q chunk (free dim of S^T matmul)
KB = 128           # k block (partition dim of S^T)
NT = T // 128      # 32 row-tiles
NQC = T // QCH     # 8 q chunks
CCH = C // 128     # 6 contraction chunks


def _build_nc(num_devices=N_CORES, replica_groups=None, dev_single=False,
              stop_after=None):
    import concourse.mybir as mybir
    import concourse.tile as tile
    from concourse import bacc

    if dev_single:
        num_devices = 1
    if replica_groups is None:
        replica_groups = [[0, 1, 2, 3], [4, 5, 6, 7]]
    phase_order = ["xT", "qkv", "vp", "attn", "proj"]
    active = phase_order if stop_after is None else \
        phase_order[:phase_order.index(stop_after) + 1]

    fp32 = mybir.dt.float32
    bf16 = mybir.dt.bfloat16

    nc = bacc.Bacc("TRN2", target_bir_lowering=False, debug=False,
                   num_devices=num_devices)
    x_in = nc.dram_tensor("x", [T, C], fp32, kind="ExternalInput")
    wqkv_in = nc.dram_tensor("wqkv", [C, 640], fp32, kind="ExternalInput")
    wp_in = nc.dram_tensor("wp", [HPC * D, C], fp32, kind="ExternalInput")
    iden_in = nc.dram_tensor("iden", [128, 128], fp32, kind="ExternalInput")
    masks_in = nc.dram_tensor("masks", [128, 128], fp32, kind="ExternalInput")
    out = nc.dram_tensor("out", [T // 4, C], fp32, kind="ExternalOutput")

    with tile.TileContext(nc) as tc:
        with tc.tile_pool(name="pers", bufs=1) as pers, \
             tc.tile_pool(name="dram", bufs=1, space="DRAM") as dram:

            # ---- constants ----
            idf = pers.tile([128, 128], fp32)
            nc.sync.dma_start(idf[:], iden_in.ap()[:])
            idb = pers.tile([128, 128], bf16)
            nc.vector.tensor_copy(idb[:], idf[:])
            with tc.tile_pool(name="mstage", bufs=2) as mstage:
                mf = mstage.tile([128, 128], fp32, tag="mf")
                nc.sync.dma_start(mf[:], masks_in.ap()[:, :])
                maskt = pers.tile([128, 128], bf16)
                nc.vector.tensor_copy(maskt[:], mf[:])

            # ---- weights ----
            wqb = []
            with tc.tile_pool(name="wstage", bufs=2) as wstage:
                for ci in range(CCH):
                    wf = wstage.tile([128, 640], fp32, tag="wf")
                    nc.sync.dma_start(wf[:], wqkv_in.ap()[ci * 128:(ci + 1) * 128, :])
                    wb = pers.tile([128, 640], bf16, name=f"wqb{ci}")
                    nc.vector.tensor_copy(wb[:], wf[:])
                    wqb.append(wb)
                wpf_a = wstage.tile([128, C], fp32, tag="wpf")
                nc.sync.dma_start(wpf_a[:], wp_in.ap()[0:128, :])
                wpb_a = pers.tile([128, C], bf16)
                nc.vector.tensor_copy(wpb_a[:], wpf_a[:])
                wpf_b = wstage.tile([64, C], fp32, tag="wpf")
                nc.sync.dma_start(wpf_b[:], wp_in.ap()[128:192, :])
                wpb_b = pers.tile([64, C], bf16)
                nc.vector.tensor_copy(wpb_b[:], wpf_b[:])

            # ---- x load + PE-transpose (cast to bf16 in the PSUM->SBUF copy) ----
            q_loc = [(0, 0), (0, 64), (2, 0)]
            k_loc = [(1, 0), (1, 64), (3, 0)]
            v_loc = [(4, 0), (4, 64), (2, 64)]
            vp = [[None] * NT for _ in range(HPC)]
            with tc.tile_pool(name="xT", bufs=1) as xtp, \
                 tc.tile_pool(name="xstage", bufs=4) as xstage, \
                 tc.tile_pool(name="xtps", bufs=4, space="PSUM") as xtps:
                xT = [xtp.tile([128, T], bf16, name=f"xT{ci}") for ci in range(CCH)]
                for ti in range(NT):
                    xf = xstage.tile([128, C], fp32, tag="xf")
                    nc.sync.dma_start(xf[:], x_in.ap()[ti * 128:(ti + 1) * 128, :])
                    for ci in range(CCH):
                        xps = xtps.tile([128, 128], fp32, tag="xps")
                        nc.tensor.transpose(
                            xps[:], xf[:, ci * 128:(ci + 1) * 128], idf[:, :])
                        nc.scalar.copy(
                            xT[ci][:, ti * 128:(ti + 1) * 128], xps[:])

                # ---- QKV^T projection ----
                # wqkv col layout (640): [q0|q1, k0|k1, q2|v2, k2|pad, v0|v1]
                qkvT = [pers.tile([128, T], bf16, name=f"qkvT{m}") for m in range(5)]
                with tc.tile_pool(name="qkvps", bufs=3, space="PSUM") as qkvps:
                    for tch in range(NQC if "qkv" in active else 0):
                        tsl = slice(tch * QCH, (tch + 1) * QCH)
                        for m in range(5):
                            ps = qkvps.tile([128, QCH], fp32, tag="qkvp")
                            for ci in range(CCH):
                                nc.tensor.matmul(
                                    ps[:],
                                    wqb[ci][:, m * 128:(m + 1) * 128],
                                    xT[ci][:, tsl],
                                    start=(ci == 0), stop=(ci == CCH - 1),
                                )
                            nc.vector.tensor_copy(qkvT[m][:, tsl], ps[:])

            # ---- V' tiles (V [kblock, 64] + ones col) ----
            with tc.tile_pool(name="vps", bufs=3, space="PSUM") as vps:
                for h in range(HPC if "vp" in active else 0):
                    vm, vo = v_loc[h]
                    for kt in range(NT):
                        tp = vps.tile([128, D], bf16, tag="vtp")
                        nc.tensor.transpose(
                            tp[:],
                            qkvT[vm][vo:vo + D, kt * 128:(kt + 1) * 128],
                            idb[vo:vo + D, vo:vo + D],
                        )
                        vpt = pers.tile([128, D + 1], bf16, name=f"vp{h}_{kt}")
                        nc.vector.tensor_copy(vpt[:, 0:D], tp[:])
                        nc.vector.memset(vpt[:, D:D + 1], 1.0)
                        vp[h][kt] = vpt

            # ---- attention (+ interleaved partial out-proj and split RS) ----
            OT_a = pers.tile([128, T], bf16)   # heads 0,1 rows
            OT_b = pers.tile([64, T], bf16)    # head 2
            send = dram.tile([T, C], bf16)
            recv1 = dram.tile([T // 8, C], bf16)
            recv2 = dram.tile([T // 8, C], bf16)
            with tc.tile_pool(name="sps", bufs=2, space="PSUM") as sps, \
                 tc.tile_pool(name="ops", bufs=2, space="PSUM") as ops, \
                 tc.tile_pool(name="tps", bufs=1, space="PSUM") as tps, \
                 tc.tile_pool(name="otps", bufs=1, space="PSUM") as otps, \
                 tc.tile_pool(name="ptp", bufs=4) as ptp, \
                 tc.tile_pool(name="ystage", bufs=3) as ystage, \
                 tc.tile_pool(name="epi", bufs=2) as epi:
                for qc in range(NQC if "attn" in active else 0):
                    nkb = (qc + 1) * (QCH // KB)
                    # per-kb: (kb, q_off, width): diag blocks (last 4) are
                    # truncated to their causal column range [128m, 512).
                    blocks = []
                    for kb in range(nkb):
                        if kb >= nkb - 4:
                            m = kb - (nkb - 4)
                            blocks.append((kb, 128 * m, QCH - 128 * m))
                        else:
                            blocks.append((kb, 0, QCH))
                    for h in range(HPC):
                        qm, qo = q_loc[h]
                        km, ko = k_loc[h]
                        op = ops.tile([D + 1, QCH], fp32, tag="op")
                        first_pv = True
                        for pi in range(0, len(blocks), 2):
                            pair = blocks[pi:pi + 2]
                            # pack second block tight if it fits before the
                            # 512 bank boundary, else at 512
                            offs = [0]
                            if len(pair) == 2:
                                w0 = pair[0][2]
                                offs.append(w0 if w0 + pair[1][2] <= 512 else 512)
                            sp = sps.tile([128, 2 * QCH], fp32, tag="sp")
                            for (kb, qoff, w), po in zip(pair, offs):
                                nc.tensor.matmul(
                                    sp[:, po:po + w],
                                    qkvT[km][ko:ko + D, kb * KB:(kb + 1) * KB],
                                    qkvT[qm][qo:qo + D,
                                             qc * QCH + qoff:(qc + 1) * QCH],
                                    start=True, stop=True,
                                )
                            pt = ptp.tile([128, 2 * QCH], bf16, tag="pt")
                            lastw = offs[-1] + pair[-1][2]
                            nc.scalar.activation(
                                pt[:, 0:lastw], sp[:, 0:lastw],
                                mybir.ActivationFunctionType.Exp, scale=0.125)
                            for (kb, qoff, w), po in zip(pair, offs):
                                if qoff or w < QCH or kb == nkb - 4:
                                    # diag block: triangle sits in its first
                                    # 128 columns
                                    nc.vector.tensor_mul(
                                        pt[:, po:po + 128], pt[:, po:po + 128],
                                        maskt[:, :])
                                nc.tensor.matmul(
                                    op[:, qoff:QCH], vp[h][kb][:],
                                    pt[:, po:po + w],
                                    start=first_pv and qoff == 0,
                                    stop=(kb == nkb - 1),
                                )
                                if qoff == 0:
                                    first_pv = False
                        # normalize + transpose into OT
                        ob = epi.tile([D + 1, QCH], fp32, tag="ob")
                        nc.vector.tensor_copy(ob[:], op[:])
                        for sb in range(QCH // 128):
                            col = qc * QCH + sb * 128
                            tp = tps.tile([128, D + 1], fp32, tag="tp")
                            nc.tensor.transpose(
                                tp[:], ob[:, sb * 128:(sb + 1) * 128],
                                idf[0:D + 1, 0:D + 1])
                            recip = epi.tile([128, 1], fp32, tag="recip")
                            nc.vector.reciprocal(recip[:], tp[:, D:D + 1])
                            onorm = epi.tile([128, D], bf16, tag="onorm")
                            nc.vector.tensor_scalar_mul(onorm[:], tp[:, 0:D], recip[:])
                            otp = otps.tile([D, 128], bf16, tag="otp")
                            nc.tensor.transpose(otp[:], onorm[:], idb[:, :])
                            if h < 2:
                                nc.vector.tensor_copy(
                                    OT_a[h * D:(h + 1) * D, col:col + 128], otp[:])
                            else:
                                nc.vector.tensor_copy(
                                    OT_b[:, col:col + 128], otp[:])
                    if "proj" in active:
                        for tt in range(4 * qc, 4 * qc + 4):
                            csl = slice(tt * 128, (tt + 1) * 128)
                            pA = ops.tile([128, 512], fp32, tag="op")
                            pB = ops.tile([128, 256], fp32, tag="op")
                            nc.tensor.matmul(pA[:], OT_a[:, csl], wpb_a[:, 0:512],
                                             start=True, stop=False)
                            nc.tensor.matmul(pA[:], OT_b[:, csl], wpb_b[:, 0:512],
                                             start=False, stop=True)
                            nc.tensor.matmul(pB[:], OT_a[:, csl], wpb_a[:, 512:768],
                                             start=True, stop=False)
                            nc.tensor.matmul(pB[:], OT_b[:, csl], wpb_b[:, 512:768],
                                             start=False, stop=True)
                            ysb = ystage.tile([128, C], bf16, tag="ysb")
                            nc.vector.tensor_copy(ysb[:, 0:512], pA[:])
                            nc.vector.tensor_copy(ysb[:, 512:768], pB[:])
                            nc.sync.dma_start(send[csl, :], ysb[:])
                        if qc == 3:
                            if dev_single:
                                nc.sync.dma_start(recv1[:, :], send[0:T // 8, :])
                            else:
                                nc.gpsimd.collective_compute(
                                    "ReduceScatter", mybir.AluOpType.add,
                                    replica_groups=replica_groups,
                                    ins=[send[0:T // 2, :].opt()],
                                    outs=[recv1.opt()])
                            nc.gpsimd.dma_start(out.ap()[0:T // 8, :], recv1[:, :])
                        if qc == 7:
                            if dev_single:
                                nc.sync.dma_start(recv2[:, :], send[T // 2:T // 2 + T // 8, :])
                            else:
                                nc.gpsimd.collective_compute(
                                    "ReduceScatter", mybir.AluOpType.add,
                                    replica_groups=replica_groups,
                                    ins=[send[T // 2:T, :].opt()],
                                    outs=[recv2.opt()])
                            nc.gpsimd.dma_start(out.ap()[T // 8:T // 4, :], recv2[:, :])

    nc.compile()
    return nc


def make_core_inputs(x, w_attn, w_proj, core):
    """Build the per-core input dict from full problem inputs."""
    b, hg = core // 4, core % 4
    hs = HPC * hg
    q = [w_attn[:, (hs + j) * D:(hs + j + 1) * D] for j in range(HPC)]
    k = [w_attn[:, C + (hs + j) * D:C + (hs + j + 1) * D] for j in range(HPC)]
    v = [w_attn[:, 2 * C + (hs + j) * D:2 * C + (hs + j + 1) * D] for j in range(HPC)]
    pad = np.zeros((C, D), dtype=np.float32)
    # col layout: [q0|q1, k0|k1, q2|v2, k2|pad, v0|v1]
    wqkv = np.concatenate([q[0], q[1], k[0], k[1], q[2], v[2], k[2], pad, v[0], v[1]],
                          axis=1)
    wp = w_proj[hs * D:(hs + HPC) * D, :]
    iden = np.eye(128, dtype=np.float32)
    masks = (np.arange(128)[:, None] <= np.arange(128)[None, :]).astype(np.float32)
    return {
        "x": np.ascontiguousarray(x[b]),
        "wqkv": np.ascontiguousarray(wqkv),
        "wp": np.ascontiguousarray(wp),
        "iden": iden,
        "masks": masks,
    }


_CACHE = {}


class _SpmdRunner:
    """Executes the prebuilt Bass module on the 8 axon NeuronCores via PJRT
    (mirrors concourse.bass2jax.run_bass_via_pjrt's multi-core path, but jits
    once so repeated calls are cheap)."""

    def __init__(self, nc, n_cores=N_CORES):
        import jax
        from jax.sharding import Mesh, PartitionSpec
        try:
            from jax import shard_map
            def _shard_map(f, mesh, in_specs, out_specs):
                return shard_map(f, mesh=mesh, in_specs=in_specs,
                                 out_specs=out_specs, check_vma=False)
        except ImportError:
            from jax.experimental.shard_map import shard_map
            def _shard_map(f, mesh, in_specs, out_specs):
                return shard_map(f, mesh=mesh, in_specs=in_specs,
                                 out_specs=out_specs, check_rep=False)
        import concourse.mybir as mybir
        from concourse.bass2jax import (_bass_exec_p, install_neuronx_cc_hook,
                                        partition_id_tensor)

        install_neuronx_cc_hook()
        self.nc = nc
        self.n_cores = n_cores
        partition_name = (nc.partition_id_tensor.name
                          if nc.partition_id_tensor else None)
        in_names, out_names, out_avals, zero_outs = [], [], [], []
        for alloc in nc.m.functions[0].allocations:
            if not isinstance(alloc, mybir.MemoryLocationSet):
                continue
            name = alloc.memorylocations[0].name
            if alloc.kind == "ExternalInput":
                if name != partition_name:
                    in_names.append(name)
            elif alloc.kind == "ExternalOutput":
                out_names.append(name)
                shape = tuple(alloc.tensor_shape)
                dtype = mybir.dt.np(alloc.dtype)
                out_avals.append(jax.core.ShapedArray(shape, dtype))
                zero_outs.append(np.zeros(shape, dtype))
        self.in_names, self.out_names = in_names, out_names
        self.out_avals, self.zero_outs = tuple(out_avals), zero_outs
        n_params, n_outs = len(in_names), len(out_avals)
        all_in = list(in_names) + list(out_names)
        if partition_name is not None:
            all_in.append(partition_name)

        def _body(*args):
            operands = list(args)
            if partition_name is not None:
                operands.append(partition_id_tensor())
            outs = _bass_exec_p.bind(
                *operands,
                out_avals=self.out_avals,
                in_names=tuple(all_in),
                out_names=tuple(out_names),
                lowering_input_output_aliases=(),
                sim_require_finite=True,
                sim_require_nnan=True,
                nc=nc,
            )
            return tuple(outs)

        devices = jax.devices()[:n_cores]
        self.mesh = Mesh(np.asarray(devices), ("core",))
        in_specs = (PartitionSpec("core"),) * (n_params + n_outs)
        out_specs = (PartitionSpec("core"),) * n_outs
        self.fn = jax.jit(
            _shard_map(_body, self.mesh, in_specs, out_specs),
            donate_argnums=tuple(range(n_params, n_params + n_outs)),
            keep_unused=True,
        )

    def concat_inputs(self, in_maps):
        return [
            np.concatenate([np.asarray(in_maps[c][name])
                            for c in range(self.n_cores)], axis=0)
            for name in self.in_names
        ]

    def zeros(self):
        return [np.zeros((self.n_cores * z.shape[0], *z.shape[1:]), z.dtype)
                for z in self.zero_outs]

    def __call__(self, concat_in, out_bufs=None):
        if out_bufs is None:
            out_bufs = self.zeros()
        return self.fn(*concat_in, *out_bufs)

    def split_outputs(self, out_arrs):
        res = []
        for c in range(self.n_cores):
            res.append({
                name: np.asarray(out_arrs[c * self.out_avals[i].shape[0]:
                                          (c + 1) * self.out_avals[i].shape[0]]
                                 if False else out_arrs[i]).reshape(
                    self.n_cores, *self.out_avals[i].shape)[c]
                for i, name in enumerate(self.out_names)})
        return res


def _get_runner():
    if "runner" not in _CACHE:
        nc = _build_nc()
        _CACHE["runner"] = _SpmdRunner(nc)
    return _CACHE["runner"]


def kernel(x, w_attn, w_proj):
    import jax
    runner = _get_runner()
    in_maps = [make_core_inputs(x, w_attn, w_proj, c) for c in range(N_CORES)]
    ci = runner.concat_inputs(in_maps)
    r = runner(ci)
    jax.block_until_ready(r)
    res = runner.split_outputs(r)
    out = np.empty((B, T, C), dtype=np.float32)
    for c in range(N_CORES):
        b, j = c // 4, c % 4
        out[b, 512 * j:512 * (j + 1), :] = res[c]["out"][0:512]
        out[b, 2048 + 512 * j:2048 + 512 * (j + 1), :] = res[c]["out"][512:1024]
    return out


# revision 19
# speedup vs baseline: 5.9243x; 5.9243x over previous
"""Causal self-attention (B=2, T=4096, C=768, H=12, D=64) on 8 TRN2 NeuronCores.

Sharding: tensor-parallel over heads x data-parallel over batch.
  core i (i in 0..7): batch b = i // 4, heads hs..hs+2 where hs = 3 * (i % 4).

Per-core kernel:
  1. x[b] -> bf16 -> on-chip DMA-transpose -> x^T [768, 4096]
  2. QKV^T projection for its 3 heads (bf16 matmuls, f32 PSUM accumulation)
  3. causal attention per head: S^T = K^T.T @ Q^T blocks, exp (no max-sub:
     logits are O(1) so exp is safe), diagonal-block masking, P@V' with an
     appended ones-column producing row sums, then normalization
  4. partial output projection (only its heads' rows of w_proj)
  5. ReduceScatter(add) over each 4-core group scattering T -> each core owns
     a 1024-row chunk of the final output.

Host side only shards/concatenates and pre-slices weight columns.
"""

import numpy as np

B, T, C, H, D = 2, 4096, 768, 12, 64
N_CORES = 8
HPC = 3            # heads per core
QCH = 512          # q chunk (free dim of S^T matmul)
KB = 128           # k block (partition dim of S^T)
NT = T // 128      # 32 row-tiles
NQC = T // QCH     # 8 q chunks
CCH = C // 128     # 6 contraction chunks


def _build_nc(num_devices=N_CORES, replica_groups=None, dev_single=False,
              stop_after=None):
    import concourse.mybir as mybir
    import concourse.tile as tile
    from concourse import bacc

    if dev_single:
        num_devices = 1
    if replica_groups is None:
        replica_groups = [[0, 1, 2, 3], [4, 5, 6, 7]]
    phase_order = ["xT", "qkv", "vp", "attn", "proj"]
    active = phase_order if stop_after is None else \
        phase_order[:phase_order.index(stop_after) + 1]

    fp32 = mybir.dt.float32
    bf16 = mybir.dt.bfloat16

    nc = bacc.Bacc("TRN2", target_bir_lowering=False, debug=False,
                   num_devices=num_devices)
    x_in = nc.dram_tensor("x", [T, C], fp32, kind="ExternalInput")
    wqkv_in = nc.dram_tensor("wqkv", [C, 640], fp32, kind="ExternalInput")
    wp_in = nc.dram_tensor("wp", [HPC * D, C], fp32, kind="ExternalInput")
    iden_in = nc.dram_tensor("iden", [128, 128], fp32, kind="ExternalInput")
    masks_in = nc.dram_tensor("masks", [128, 128], fp32, kind="ExternalInput")
    out = nc.dram_tensor("out", [T // 4, C], fp32, kind="ExternalOutput")

    with tile.TileContext(nc) as tc:
        with tc.tile_pool(name="pers", bufs=1) as pers, \
             tc.tile_pool(name="dram", bufs=1, space="DRAM") as dram:

            # ---- constants ----
            idf = pers.tile([128, 128], fp32)
            nc.sync.dma_start(idf[:], iden_in.ap()[:])
            idb = pers.tile([128, 128], bf16)
            nc.vector.tensor_copy(idb[:], idf[:])
            with tc.tile_pool(name="mstage", bufs=2) as mstage:
                mf = mstage.tile([128, 128], fp32, tag="mf")
                nc.sync.dma_start(mf[:], masks_in.ap()[:, :])
                maskt = pers.tile([128, 128], bf16)
                nc.vector.tensor_copy(maskt[:], mf[:])

            # ---- weights ----
            wqb = []
            with tc.tile_pool(name="wstage", bufs=2) as wstage:
                for ci in range(CCH):
                    wf = wstage.tile([128, 640], fp32, tag="wf")
                    nc.sync.dma_start(wf[:], wqkv_in.ap()[ci * 128:(ci + 1) * 128, :])
                    wb = pers.tile([128, 640], bf16, name=f"wqb{ci}")
                    nc.vector.tensor_copy(wb[:], wf[:])
                    wqb.append(wb)
                wpf_a = wstage.tile([128, C], fp32, tag="wpf")
                nc.sync.dma_start(wpf_a[:], wp_in.ap()[0:128, :])
                wpb_a = pers.tile([128, C], bf16)
                nc.vector.tensor_copy(wpb_a[:], wpf_a[:])
                wpf_b = wstage.tile([64, C], fp32, tag="wpf")
                nc.sync.dma_start(wpf_b[:], wp_in.ap()[128:192, :])
                wpb_b = pers.tile([64, C], bf16)
                nc.vector.tensor_copy(wpb_b[:], wpf_b[:])

            # ---- x load + PE-transpose (cast to bf16 in the PSUM->SBUF copy) ----
            q_loc = [(0, 0), (0, 64), (2, 0)]
            k_loc = [(1, 0), (1, 64), (3, 0)]
            v_loc = [(4, 0), (4, 64), (2, 64)]
            vp = [[None] * NT for _ in range(HPC)]
            with tc.tile_pool(name="xT", bufs=1) as xtp, \
                 tc.tile_pool(name="xstage", bufs=4) as xstage, \
                 tc.tile_pool(name="xtps", bufs=4, space="PSUM") as xtps:
                xT = [xtp.tile([128, T], bf16, name=f"xT{ci}") for ci in range(CCH)]
                for tg in range(NT // 4):
                    xfs = []
                    for ti in range(4 * tg, 4 * tg + 4):
                        xff = xstage.tile([128, C], fp32, tag="xff")
                        nc.sync.dma_start(xff[:], x_in.ap()[ti * 128:(ti + 1) * 128, :])
                        xf = xstage.tile([128, C], bf16, tag="xf")
                        nc.vector.tensor_copy(xf[:], xff[:])
                        xfs.append(xf)
                    for ci in range(CCH):
                        xps = xtps.tile([128, 512], bf16, tag="xps")
                        for j in range(4):
                            nc.tensor.transpose(
                                xps[:, j * 128:(j + 1) * 128],
                                xfs[j][:, ci * 128:(ci + 1) * 128], idb[:, :])
                        nc.scalar.copy(
                            xT[ci][:, tg * 512:(tg + 1) * 512], xps[:])

                # ---- QKV^T projection ----
                # wqkv col layout (640): [q0|q1, k0|k1, q2|v2, k2|pad, v0|v1]
                qkvT = [pers.tile([128, T], bf16, name=f"qkvT{m}") for m in range(5)]
                with tc.tile_pool(name="qkvps", bufs=3, space="PSUM") as qkvps:
                    for tch in range(NQC if "qkv" in active else 0):
                        tsl = slice(tch * QCH, (tch + 1) * QCH)
                        for m in range(5):
                            ps = qkvps.tile([128, QCH], fp32, tag="qkvp")
                            for ci in range(CCH):
                                nc.tensor.matmul(
                                    ps[:],
                                    wqb[ci][:, m * 128:(m + 1) * 128],
                                    xT[ci][:, tsl],
                                    start=(ci == 0), stop=(ci == CCH - 1),
                                )
                            nc.vector.tensor_copy(qkvT[m][:, tsl], ps[:])

            # ---- V' tiles (V [kblock, 64] + ones col), merged per head ----
            vpbuf = [pers.tile([128, NT * (D + 1)], bf16, name=f"vpbuf{h}")
                     for h in range(HPC)]
            with tc.tile_pool(name="vps", bufs=3, space="PSUM") as vps:
                for h in range(HPC if "vp" in active else 0):
                    vm, vo = v_loc[h]
                    for kg in range(NT // 4):
                        tp = vps.tile([128, 4 * D], bf16, tag="vtp")
                        for j in range(4):
                            kt = 4 * kg + j
                            nc.tensor.transpose(
                                tp[:, j * D:(j + 1) * D],
                                qkvT[vm][vo:vo + D, kt * 128:(kt + 1) * 128],
                                idb[vo:vo + D, vo:vo + D],
                            )
                        dst = vpbuf[h][:, 4 * kg * (D + 1):(4 * kg + 4) * (D + 1)]
                        dst3 = dst.rearrange("p (g d) -> p g d", d=D + 1)
                        src3 = tp[:].rearrange("p (g d) -> p g d", d=D)
                        nc.vector.tensor_copy(dst3[:, :, 0:D], src3[:])
                        nc.vector.memset(dst3[:, :, D:D + 1], 1.0)
                    for kt in range(NT):
                        vp[h][kt] = vpbuf[h][:, kt * (D + 1):(kt + 1) * (D + 1)]

            # ---- attention (+ interleaved partial out-proj and split RS) ----
            OT_a = pers.tile([128, T], bf16)   # heads 0,1 rows
            OT_b = pers.tile([64, T], bf16)    # head 2
            send = dram.tile([T, C], bf16)
            recv1 = dram.tile([T // 8, C], bf16)
            recv2 = dram.tile([T // 8, C], bf16)
            with tc.tile_pool(name="sps", bufs=2, space="PSUM") as sps, \
                 tc.tile_pool(name="ops", bufs=2, space="PSUM") as ops, \
                 tc.tile_pool(name="tps", bufs=1, space="PSUM") as tps, \
                 tc.tile_pool(name="otps", bufs=1, space="PSUM") as otps, \
                 tc.tile_pool(name="ptp", bufs=4) as ptp, \
                 tc.tile_pool(name="ystage", bufs=3) as ystage, \
                 tc.tile_pool(name="epi", bufs=2) as epi:
                for qc in range(NQC if "attn" in active else 0):
                    nkb = (qc + 1) * (QCH // KB)
                    # per-kb: (kb, q_off, width): diag blocks (last 4) are
                    # truncated to their causal column range [128m, 512).
                    blocks = []
                    for kb in range(nkb):
                        if kb >= nkb - 4:
                            m = kb - (nkb - 4)
                            blocks.append((kb, 128 * m, QCH - 128 * m))
                        else:
                            blocks.append((kb, 0, QCH))
                    for h in range(HPC):
                        qm, qo = q_loc[h]
                        km, ko = k_loc[h]
                        op = ops.tile([D + 1, QCH], fp32, tag="op")
                        first_pv = True
                        for pi in range(0, len(blocks), 2):
                            pair = blocks[pi:pi + 2]
                            # pack second block tight if it fits before the
                            # 512 bank boundary, else at 512
                            offs = [0]
                            if len(pair) == 2:
                                w0 = pair[0][2]
                                offs.append(w0 if w0 + pair[1][2] <= 512 else 512)
                            sp = sps.tile([128, 2 * QCH], fp32, tag="sp")
                            for (kb, qoff, w), po in zip(pair, offs):
                                nc.tensor.matmul(
                                    sp[:, po:po + w],
                                    qkvT[km][ko:ko + D, kb * KB:(kb + 1) * KB],
                                    qkvT[qm][qo:qo + D,
                                             qc * QCH + qoff:(qc + 1) * QCH],
                                    start=True, stop=True,
                                )
                            pt = ptp.tile([128, 2 * QCH], bf16, tag="pt")
                            lastw = offs[-1] + pair[-1][2]
                            nc.scalar.activation(
                                pt[:, 0:lastw], sp[:, 0:lastw],
                                mybir.ActivationFunctionType.Exp, scale=0.125)
                            for (kb, qoff, w), po in zip(pair, offs):
                                if qoff or w < QCH or kb == nkb - 4:
                                    # diag block: triangle sits in its first
                                    # 128 columns
                                    nc.vector.tensor_mul(
                                        pt[:, po:po + 128], pt[:, po:po + 128],
                                        maskt[:, :])
                                nc.tensor.matmul(
                                    op[:, qoff:QCH], vp[h][kb],
                                    pt[:, po:po + w],
                                    start=first_pv and qoff == 0,
                                    stop=(kb == nkb - 1),
                                )
                                if qoff == 0:
                                    first_pv = False
                        # normalize + transpose into OT
                        ob = epi.tile([D + 1, QCH], fp32, tag="ob")
                        nc.vector.tensor_copy(ob[:], op[:])
                        for sb in range(QCH // 128):
                            col = qc * QCH + sb * 128
                            tp = tps.tile([128, D + 1], fp32, tag="tp")
                            nc.tensor.transpose(
                                tp[:], ob[:, sb * 128:(sb + 1) * 128],
                                idf[0:D + 1, 0:D + 1])
                            recip = epi.tile([128, 1], fp32, tag="recip")
                            nc.vector.reciprocal(recip[:], tp[:, D:D + 1])
                            onorm = epi.tile([128, D], bf16, tag="onorm")
                            nc.vector.tensor_scalar_mul(onorm[:], tp[:, 0:D], recip[:])
                            otp = otps.tile([D, 128], bf16, tag="otp")
                            nc.tensor.transpose(otp[:], onorm[:], idb[:, :])
                            if h < 2:
                                nc.vector.tensor_copy(
                                    OT_a[h * D:(h + 1) * D, col:col + 128], otp[:])
                            else:
                                nc.vector.tensor_copy(
                                    OT_b[:, col:col + 128], otp[:])
                    if "proj" in active:
                        for tt in range(4 * qc, 4 * qc + 4):
                            csl = slice(tt * 128, (tt + 1) * 128)
                            pA = ops.tile([128, 512], fp32, tag="op")
                            pB = ops.tile([128, 256], fp32, tag="op")
                            nc.tensor.matmul(pA[:], OT_a[:, csl], wpb_a[:, 0:512],
                                             start=True, stop=False)
                            nc.tensor.matmul(pA[:], OT_b[:, csl], wpb_b[:, 0:512],
                                             start=False, stop=True)
                            nc.tensor.matmul(pB[:], OT_a[:, csl], wpb_a[:, 512:768],
                                             start=True, stop=False)
                            nc.tensor.matmul(pB[:], OT_b[:, csl], wpb_b[:, 512:768],
                                             start=False, stop=True)
                            ysb = ystage.tile([128, C], bf16, tag="ysb")
                            nc.vector.tensor_copy(ysb[:, 0:512], pA[:])
                            nc.vector.tensor_copy(ysb[:, 512:768], pB[:])
                            nc.sync.dma_start(send[csl, :], ysb[:])
                        if qc == 3:
                            if dev_single:
                                nc.sync.dma_start(recv1[:, :], send[0:T // 8, :])
                            else:
                                nc.gpsimd.collective_compute(
                                    "ReduceScatter", mybir.AluOpType.add,
                                    replica_groups=replica_groups,
                                    ins=[send[0:T // 2, :].opt()],
                                    outs=[recv1.opt()])
                            nc.gpsimd.dma_start(out.ap()[0:T // 8, :], recv1[:, :])
                        if qc == 7:
                            if dev_single:
                                nc.sync.dma_start(recv2[:, :], send[T // 2:T // 2 + T // 8, :])
                            else:
                                nc.gpsimd.collective_compute(
                                    "ReduceScatter", mybir.AluOpType.add,
                                    replica_groups=replica_groups,
                                    ins=[send[T // 2:T, :].opt()],
                                    outs=[recv2.opt()])
                            nc.gpsimd.dma_start(out.ap()[T // 8:T // 4, :], recv2[:, :])

    nc.compile()
    return nc


def make_core_inputs(x, w_attn, w_proj, core):
    """Build the per-core input dict from full problem inputs."""
    b, hg = core // 4, core % 4
    hs = HPC * hg
    q = [w_attn[:, (hs + j) * D:(hs + j + 1) * D] for j in range(HPC)]
    k = [w_attn[:, C + (hs + j) * D:C + (hs + j + 1) * D] for j in range(HPC)]
    v = [w_attn[:, 2 * C + (hs + j) * D:2 * C + (hs + j + 1) * D] for j in range(HPC)]
    pad = np.zeros((C, D), dtype=np.float32)
    # col layout: [q0|q1, k0|k1, q2|v2, k2|pad, v0|v1]
    wqkv = np.concatenate([q[0], q[1], k[0], k[1], q[2], v[2], k[2], pad, v[0], v[1]],
                          axis=1)
    wp = w_proj[hs * D:(hs + HPC) * D, :]
    iden = np.eye(128, dtype=np.float32)
    masks = (np.arange(128)[:, None] <= np.arange(128)[None, :]).astype(np.float32)
    return {
        "x": np.ascontiguousarray(x[b]),
        "wqkv": np.ascontiguousarray(wqkv),
        "wp": np.ascontiguousarray(wp),
        "iden": iden,
        "masks": masks,
    }


_CACHE = {}


class _SpmdRunner:
    """Executes the prebuilt Bass module on the 8 axon NeuronCores via PJRT
    (mirrors concourse.bass2jax.run_bass_via_pjrt's multi-core path, but jits
    once so repeated calls are cheap)."""

    def __init__(self, nc, n_cores=N_CORES):
        import jax
        from jax.sharding import Mesh, PartitionSpec
        try:
            from jax import shard_map
            def _shard_map(f, mesh, in_specs, out_specs):
                return shard_map(f, mesh=mesh, in_specs=in_specs,
                                 out_specs=out_specs, check_vma=False)
        except ImportError:
            from jax.experimental.shard_map import shard_map
            def _shard_map(f, mesh, in_specs, out_specs):
                return shard_map(f, mesh=mesh, in_specs=in_specs,
                                 out_specs=out_specs, check_rep=False)
        import concourse.mybir as mybir
        from concourse.bass2jax import (_bass_exec_p, install_neuronx_cc_hook,
                                        partition_id_tensor)

        install_neuronx_cc_hook()
        self.nc = nc
        self.n_cores = n_cores
        partition_name = (nc.partition_id_tensor.name
                          if nc.partition_id_tensor else None)
        in_names, out_names, out_avals, zero_outs = [], [], [], []
        for alloc in nc.m.functions[0].allocations:
            if not isinstance(alloc, mybir.MemoryLocationSet):
                continue
            name = alloc.memorylocations[0].name
            if alloc.kind == "ExternalInput":
                if name != partition_name:
                    in_names.append(name)
            elif alloc.kind == "ExternalOutput":
                out_names.append(name)
                shape = tuple(alloc.tensor_shape)
                dtype = mybir.dt.np(alloc.dtype)
                out_avals.append(jax.core.ShapedArray(shape, dtype))
                zero_outs.append(np.zeros(shape, dtype))
        self.in_names, self.out_names = in_names, out_names
        self.out_avals, self.zero_outs = tuple(out_avals), zero_outs
        n_params, n_outs = len(in_names), len(out_avals)
        all_in = list(in_names) + list(out_names)
        if partition_name is not None:
            all_in.append(partition_name)

        def _body(*args):
            operands = list(args)
            if partition_name is not None:
                operands.append(partition_id_tensor())
            outs = _bass_exec_p.bind(
                *operands,
                out_avals=self.out_avals,
                in_names=tuple(all_in),
                out_names=tuple(out_names),
                lowering_input_output_aliases=(),
                sim_require_finite=True,
                sim_require_nnan=True,
                nc=nc,
            )
            return tuple(outs)

        devices = jax.devices()[:n_cores]
        self.mesh = Mesh(np.asarray(devices), ("core",))
        in_specs = (PartitionSpec("core"),) * (n_params + n_outs)
        out_specs = (PartitionSpec("core"),) * n_outs
        self.fn = jax.jit(
            _shard_map(_body, self.mesh, in_specs, out_specs),
            donate_argnums=tuple(range(n_params, n_params + n_outs)),
            keep_unused=True,
        )

    def concat_inputs(self, in_maps):
        return [
            np.concatenate([np.asarray(in_maps[c][name])
                            for c in range(self.n_cores)], axis=0)
            for name in self.in_names
        ]

    def zeros(self):
        return [np.zeros((self.n_cores * z.shape[0], *z.shape[1:]), z.dtype)
                for z in self.zero_outs]

    def __call__(self, concat_in, out_bufs=None):
        if out_bufs is None:
            out_bufs = self.zeros()
        return self.fn(*concat_in, *out_bufs)

    def split_outputs(self, out_arrs):
        res = []
        for c in range(self.n_cores):
            res.append({
                name: np.asarray(out_arrs[i]).reshape(
                    self.n_cores, *self.out_avals[i].shape)[c]
                for i, name in enumerate(self.out_names)})
        return res


def _get_runner():
    if "runner" not in _CACHE:
        nc = _build_nc()
        _CACHE["runner"] = _SpmdRunner(nc)
    return _CACHE["runner"]


def kernel(x, w_attn, w_proj):
    import jax
    x = np.asarray(x, dtype=np.float32)
    w_attn = np.asarray(w_attn, dtype=np.float32)
    w_proj = np.asarray(w_proj, dtype=np.float32)
    runner = _get_runner()
    in_maps = [make_core_inputs(x, w_attn, w_proj, c) for c in range(N_CORES)]
    ci = runner.concat_inputs(in_maps)
    r = runner(ci)
    jax.block_until_ready(r)
    res = runner.split_outputs(r)
    out = np.empty((B, T, C), dtype=np.float32)
    for c in range(N_CORES):
        b, j = c // 4, c % 4
        out[b, 512 * j:512 * (j + 1), :] = res[c]["out"][0:512]
        out[b, 2048 + 512 * j:2048 + 512 * (j + 1), :] = res[c]["out"][512:1024]
    return out
